# revision 1
# baseline (speedup 1.0000x reference)
"""CoCa image-tower kernel for 8 Trainium2 NeuronCores.

Strategy (SPMD, one program on all 8 cores):
  - core c = (batch b = c//2, chunk = c%2).  Each core conv-embeds its own
    200 images and runs the 4 "uni" transformer layers on its stream.
  - pair AllGather (cores 2b, 2b+1) exchanges the two streams, so both
    cores of a pair hold (a_b, nrm_b).
  - every core then runs the 4 (mm PTB + cross-attention) layers on a_b
    with nrm_b as context (pairs compute redundantly) and the logit head.
  - host: output row b is taken from core 2b.

Numerics: matmuls in bf16 (weights pre-cast/reordered on host, LN gains
absorbed into the following weight matrix), fp32 residual stream and
layer-norm statistics, fp32 PSUM accumulation everywhere.
"""

import numpy as np
import ml_dtypes

import concourse.bass as bass
import concourse.mybir as mybir
import concourse.tile as tile
from contextlib import ExitStack
from concourse.alu_op_type import AluOpType
from concourse.bass_utils import run_bass_kernel_spmd

AF = mybir.ActivationFunctionType
AX = mybir.AxisListType
BF16 = mybir.dt.bfloat16
F32 = mybir.dt.float32
NPBF = ml_dtypes.bfloat16

# ---------------------------------------------------------------- constants
B = 4
S = 200          # images per (batch, chunk)
D = 386
H = 8
DH = 64
FF = 1544
N = 201          # tokens (S + cls)
NC = 8
K27 = 27         # conv contraction (3 cin * 3x3)
NPOS = 361       # 19*19 conv output positions
FFP = 1664       # ff width padded to 13*128
FUSED_PTB = 512 + 128 + FFP + FFP      # 3968 = 31*128
FUSED_CA = 512 + FFP + FFP             # 3840 = 30*128
D_TILES = [(0, 97), (97, 97), (194, 96), (290, 96)]
TOK_TILES = [(0, 128), (128, 73)]      # token tiling (incl cls)
IMG_TILES = [(0, 128), (128, 72)]      # image tiling (tokens w/o cls)
EPS = 1e-5
REPLICA_PAIRS = [[0, 1], [2, 3], [4, 5], [6, 7]]

_COMPILED = {}


# ---------------------------------------------------------------- host prep
def _im2col(imgs):
    """[n,3,40,40] f32 -> [27, n*361] bf16 rows ordered (c, ky, kx)."""
    v = np.lib.stride_tricks.sliding_window_view(imgs, (3, 3), axis=(2, 3))
    v = v[:, :, ::2, ::2]                     # [n,3,19,19,3,3]
    v = v.transpose(1, 4, 5, 0, 2, 3).reshape(K27, -1)
    return np.ascontiguousarray(v).astype(NPBF)


def _ktile_cat(w):
    """[386, X] -> [97, 4*X]: D-tiles (97,97,96,96) side by side, zero pad."""
    X = w.shape[1]
    out = np.zeros((97, 4 * X), w.dtype)
    for i, (off, sz) in enumerate(D_TILES):
        out[:sz, i * X:(i + 1) * X] = w[off:off + sz]
    return out


def _rowtile_cat(w, p=128):
    """[R, X] -> [p, (R/p)*X]: row tiles side by side."""
    r, x = w.shape
    nt = r // p
    assert nt * p == r
    return np.concatenate([w[i * p:(i + 1) * p] for i in range(nt)], axis=1)


def _pad_ff_rows(w):
    """[1544, 386] -> [1664, 386] zero-padded rows."""
    return np.concatenate([w, np.zeros((FFP - FF, w.shape[1]), w.dtype)], 0)


def _prep_ptb(g, wf, wo, wff):
    wf = g[:, None] * wf
    q = wf[:, :512]
    kv = wf[:, 512:640]
    x1 = wf[:, 640:640 + FF]
    gate = wf[:, 640 + FF:]
    padc = np.zeros((D, FFP - FF), np.float32)
    wf2 = np.concatenate([q, kv, x1, padc, gate, padc], 1)
    assert wf2.shape[1] == FUSED_PTB
    return (
        _ktile_cat(wf2).astype(NPBF),
        _rowtile_cat(wo).astype(NPBF),
        _rowtile_cat(_pad_ff_rows(wff)).astype(NPBF),
    )


def _prep_ca(g, wq, wkv, wo, wf1, wf2_):
    wq = g[:, None] * wq * (DH ** -0.5)
    wf1 = g[:, None] * wf1
    h1 = wf1[:, :FF]
    gate = wf1[:, FF:]
    padc = np.zeros((D, FFP - FF), np.float32)
    fused = np.concatenate([wq, h1, padc, gate, padc], 1)
    assert fused.shape[1] == FUSED_CA
    return (
        _ktile_cat(fused).astype(NPBF),
        _ktile_cat(wkv).astype(NPBF),           # [97, 4*128]
        _rowtile_cat(wo).astype(NPBF),
        _rowtile_cat(_pad_ff_rows(wf2_)).astype(NPBF),
    )


def _rope_tables(scale):
    inv = 1.0 / (10000.0 ** (np.arange(0, DH, 2, dtype=np.float64) / DH))
    fr = np.arange(N, dtype=np.float64)[:, None] * inv[None, :]
    pos = np.concatenate([fr, fr], axis=-1)     # [N, 64]
    c = np.cos(pos).T * scale                   # [64, N]
    s = np.sin(pos).T * scale
    s_signed = np.concatenate([-s[:32], s[32:]], 0)
    ct = np.concatenate([c, c], 0)              # [128, N] (2 heads)
    st = np.concatenate([s_signed, s_signed], 0)
    return np.stack([ct, st]).astype(NPBF)      # [2, 128, N]


def _host_prep(inputs):
    """Build per-core in_maps from the raw reference inputs."""
    f32 = lambda a: np.asarray(a, np.float32)
    inp = {k: f32(v) for k, v in inputs.items()}

    # conv weights: W [27, 386] (rows c,ky,kx), pool scale folded in
    w27 = (inp["conv_w"].transpose(1, 2, 3, 0).reshape(K27, D) / NPOS).astype(NPBF)
    cb = (inp["conv_b"] / NPOS).astype(np.float32)

    cb4 = np.zeros((128, 4), np.float32)
    for ft, (off, sz) in enumerate(D_TILES):
        cb4[:sz, ft] = cb[off:off + sz]
    shared = {
        "convw": w27,
        "convb": cb4,
        "cls": inp["cls_token"].reshape(1, D).astype(np.float32),
        "ident": np.eye(128, dtype=NPBF),
        "identf": np.eye(128, dtype=np.float32),
        "ropeq": _rope_tables(DH ** -0.5),
        "ropek": _rope_tables(1.0),
        "logitw": (inp["logit_ln"][:, None] * inp["logit_w"]).T.reshape(1, 2 * D).astype(np.float32),
    }
    # causal mask multiplier tiles [2, 128, N]
    mask = np.zeros((2, 128, N), np.float32)
    for jt, (joff, jsz) in enumerate(TOK_TILES):
        j = joff + np.arange(128)[:, None]
        mask[jt] = (j <= np.arange(N)[None, :]) & (j < N)
    shared["maskm"] = mask.astype(NPBF)

    for pre in ("uni", "mm"):
        wfs, wos, wffs = [], [], []
        for i in range(4):
            a, b, c = _prep_ptb(
                inp[f"{pre}_ln"][i], inp[f"{pre}_fused"][i],
                inp[f"{pre}_attn_out"][i], inp[f"{pre}_ff_out"][i])
            wfs.append(a); wos.append(b); wffs.append(c)
        shared[f"{pre}_wf"] = np.stack(wfs)
        shared[f"{pre}_wo"] = np.stack(wos)
        shared[f"{pre}_wff"] = np.stack(wffs)

    cfs, ckvs, cos_, cf2s = [], [], [], []
    for i in range(4):
        a, b, c, d = _prep_ca(
            inp["ca_ln"][i], inp["ca_q"][i], inp["ca_kv"][i],
            inp["ca_out"][i], inp["ca_ff1"][i], inp["ca_ff2"][i])
        cfs.append(a); ckvs.append(b); cos_.append(c); cf2s.append(d)
    shared["ca_wf"] = np.stack(cfs)
    shared["ca_wkv"] = np.stack(ckvs)
    shared["ca_wo"] = np.stack(cos_)
    shared["ca_wff2"] = np.stack(cf2s)

    in_maps = []
    x = inp["input"]  # [4, 400, 3, 40, 40]
    for c in range(NC):
        b, chunk = c // 2, c % 2
        imgs = x[b, chunk * S:(chunk + 1) * S]
        m = dict(shared)
        m["im2col"] = _im2col(imgs)
        in_maps.append(m)
    return in_maps


# ---------------------------------------------------------------- device IR
def _split_multi_waits(nc, max_waits=1):
    """walrus here rejects >1 semaphore wait per instruction; split extras
    onto single-wait NoOps preceding the instruction."""
    for f in nc.m.functions:
        for blk in f.blocks:
            new_instrs = []
            for ins in blk.instructions:
                si = getattr(ins, "sync_info", None)
                waits = list(si.on_wait) if si is not None and si.on_wait else []
                if len(waits) > max_waits:
                    keep = waits[-max_waits:]
                    for j, w in enumerate(waits[:-max_waits]):
                        new_instrs.append(mybir.InstNoOp(
                            name=f"{ins.name}_wsplit{j}", engine=ins.engine,
                            ins=[], outs=[],
                            sync_info=mybir.SyncInfo(on_wait=[w], on_update=[])))
                    si.on_wait = keep
                new_instrs.append(ins)
            blk.instructions = new_instrs


class _Prog:
    """Holds the Tile program state while building."""

    def __init__(self, nc, tc, consts):
        self.nc = nc
        self.tc = tc
        self.c = consts          # dict of const sbuf tiles
        self.x = [None, None]    # residual stream tiles [128,386] f32
        self.nrmT = None         # context, transposed bf16 tiles
        self.pools = {}


def _layernorm_z(p, apool, spool, scrpool, out_dtype=BF16):
    """LN (no gain) of p.x -> z tiles (bf16).  Returns z list."""
    nc = p.nc
    z = []
    for tt, (toff, tsz) in enumerate(TOK_TILES):
        xm = scrpool.tile([128, D], F32, tag="ln_xm", name="ln_xm")
        mean = spool.tile([128, 1], F32, tag="ln_st", name="ln_st")
        nc.vector.tensor_reduce(mean[:tsz], p.x[tt][:tsz, :], AX.X, AluOpType.add)
        nc.vector.tensor_scalar_mul(mean[:tsz], mean[:tsz], 1.0 / D)
        nc.vector.tensor_scalar(xm[:tsz, :], p.x[tt][:tsz, :], mean[:tsz], None,
                                AluOpType.subtract)
        ssq = spool.tile([128, 1], F32, tag="ln_st2", name="ln_st2")
        scr = scrpool.tile([128, D], BF16, tag="ln_scr", name="ln_scr")
        nc.scalar.activation(scr[:tsz, :], xm[:tsz, :], AF.Square,
                             accum_out=ssq[:tsz])
        sd = spool.tile([128, 1], F32, tag="ln_st3", name="ln_st3")
        nc.vector.tensor_scalar(sd[:tsz], ssq[:tsz], 1.0 / D, EPS,
                                AluOpType.mult, AluOpType.add)
        nc.scalar.activation(sd[:tsz], sd[:tsz], AF.Sqrt)
        rstd = spool.tile([128, 1], F32, tag="ln_st4", name="ln_st4")
        nc.vector.reciprocal(rstd[:tsz], sd[:tsz])
        zt = apool.tile([128, D], out_dtype, tag="z", name="z", bufs=4)
        nc.vector.tensor_scalar(zt[:tsz, :], xm[:tsz, :], rstd[:tsz], None,
                                AluOpType.mult)
        z.append(zt)
    return z


def _transpose_tiles(p, src_tiles, apool, psA, tag, src_rows=TOK_TILES):
    """src [tok,386] bf16 tiles -> 4 x [97, N] bf16 transposed tiles."""
    nc = p.nc
    out = [apool.tile([97, N], BF16, tag=tag, name=tag, bufs=8 if tag == "zT" else 4) for _ in range(4)]
    for kt, (koff, ksz) in enumerate(D_TILES):
        for tt, (toff, tsz) in enumerate(src_rows):
            tp = psA.tile([128, 128], BF16, tag="psb", name="psb", bufs=2)
            nc.tensor.transpose(tp[:ksz, :tsz], src_tiles[tt][:tsz, koff:koff + ksz],
                                p.c["ident"][:tsz, :tsz])
            nc.any.tensor_copy(out[kt][:ksz, toff:toff + tsz], tp[:ksz, :tsz])
    return out


def _rope(p, raw, table, nrows, scrpool, out_tile):
    """Apply rotary embedding in [dh, tok] layout (all operands base 0).

    raw: sbuf bf16 [128, N] tile (rows = head dims, 64 per head);
    table: const [2,128,N] (cos, sin_signed); nrows: 64 or 128.
    Engines only allow a partition shift on single-input copies, so build
    the rotate-half shuffle with copies, then same-partition muls/adds."""
    nc = p.nc
    ct, st = table
    sh = scrpool.tile([128, N], BF16, tag="ropesh", name="ropesh")
    for h0 in range(0, nrows, 64):
        nc.vector.tensor_copy(sh[h0:h0 + 32, :], raw[h0 + 32:h0 + 64, :])
        nc.vector.tensor_copy(sh[h0 + 32:h0 + 64, :], raw[h0:h0 + 32, :])
    t1 = scrpool.tile([128, N], BF16, tag="rope1", name="rope1")
    nc.vector.tensor_tensor(t1[:nrows, :], raw[:nrows, :], ct[:nrows, :],
                            AluOpType.mult)
    nc.vector.tensor_tensor(sh[:nrows, :], sh[:nrows, :], st[:nrows, :],
                            AluOpType.mult)
    nc.vector.tensor_tensor(out_tile[:nrows, :], t1[:nrows, :],
                            sh[:nrows, :], AluOpType.add)


def _block(p, kind, wf_d, wo_d, wff_d, layer, pools, wkv_d=None):
    """One transformer block (PTB or CA), updates p.x in place."""
    nc = p.nc
    (wpool, apool, spool, scrpool, psA, psD, xpool) = pools
    is_ptb = kind == "ptb"
    fused_w = FUSED_PTB if is_ptb else FUSED_CA
    n_m = fused_w // 128

    wf = wpool.tile([97, 4 * FUSED_PTB], BF16, tag="wf", name="wf")
    nc.sync.dma_start(wf[:, :4 * fused_w], wf_d[layer, :, :])
    wo = wpool.tile([128, 4 * D], BF16, tag="wo", name="wo")
    nc.sync.dma_start(wo[:], wo_d[layer, :, :])
    wff = wpool.tile([128, 13 * D], BF16, tag="wff", name="wff")
    nc.sync.dma_start(wff[:], wff_d[layer, :, :])
    if not is_ptb:
        wkv = wpool.tile([97, 4 * 128], BF16, tag="wkv", name="wkv")
        nc.sync.dma_start(wkv[:], wkv_d[layer, :, :])

    z = _layernorm_z(p, apool, spool, scrpool)
    zT = _transpose_tiles(p, z, apool, psA, "zT")

    qT = [apool.tile([128, N], BF16, tag="qT", name="qT", bufs=8) for _ in range(4)]
    kT = apool.tile([64, N], BF16, tag="kT", name="kT", bufs=2)
    vaug = [apool.tile([128, 65], BF16, tag="vaug", name="vaug", bufs=4) for _ in range(2)]
    x1T = [apool.tile([128, N], BF16, tag="x1T", name="x1T", bufs=14) for _ in range(13)]
    swT = [apool.tile([128, N], BF16, tag="swT", name="swT", bufs=14) for _ in range(13)]

    def evict_kv(pf):
        kvraw = scrpool.tile([128, N], BF16, tag="kvraw", name="kvraw")
        nc.scalar.copy(kvraw[:], pf[:])
        if is_ptb:
            _rope(p, kvraw, (p.c["ropek"][0], p.c["ropek"][1]), 64, scrpool, kT)
        else:
            nc.any.tensor_copy(kT[:], kvraw[:64, :])
        vsb = scrpool.tile([64, N], BF16, tag="vsb", name="vsb")
        nc.vector.tensor_copy(vsb[:], kvraw[64:128, :])
        for jt, (joff, jsz) in enumerate(TOK_TILES):
            tv = psA.tile([128, 128], BF16, tag="psb", name="psb", bufs=2)
            nc.tensor.transpose(tv[:jsz, :64], vsb[:, joff:joff + jsz],
                                p.c["ident"][:64, :64])
            nc.any.tensor_copy(vaug[jt][:jsz, :64], tv[:jsz, :64])
            nc.vector.memset(vaug[jt][:jsz, 64:65], 1.0)

    # fused projection (stationary = weight tile, moving = zT)
    x1_base = 5 if is_ptb else 4
    gate_base = 18 if is_ptb else 17
    for m in range(n_m):
        pf = psA.tile([128, N], F32, tag="ps", name="ps")
        for kt, (koff, ksz) in enumerate(D_TILES):
            nc.tensor.matmul(pf[:, :],
                             lhsT=wf[:ksz, kt * fused_w + m * 128:
                                     kt * fused_w + (m + 1) * 128],
                             rhs=zT[kt][:ksz, :],
                             start=(kt == 0), stop=(kt == 3))
        if m < 4:  # q heads
            if is_ptb:
                qraw = scrpool.tile([128, N], BF16, tag="qraw", name="qraw")
                nc.scalar.copy(qraw[:], pf[:])
                _rope(p, qraw, (p.c["ropeq"][0], p.c["ropeq"][1]), 128, scrpool,
                      qT[m])
            else:
                nc.scalar.copy(qT[m][:], pf[:])
        elif is_ptb and m == 4:
            evict_kv(pf)
        elif x1_base <= m < gate_base:
            nc.scalar.copy(x1T[m - x1_base][:], pf[:])
        else:
            j = m - gate_base
            gs = scrpool.tile([128, N], BF16, tag="gsil", name="gsil")
            nc.scalar.activation(gs[:], pf[:], AF.Silu)
            nc.vector.tensor_tensor(swT[j][:], gs[:], x1T[j][:], AluOpType.mult)

    if not is_ptb:  # kv from context
        pkv = psA.tile([128, N], F32, tag="ps", name="ps")
        for kt, (koff, ksz) in enumerate(D_TILES):
            nc.tensor.matmul(pkv[:, :],
                             lhsT=wkv[:ksz, kt * 128:(kt + 1) * 128],
                             rhs=p.nrmT[kt][:ksz, :],
                             start=(kt == 0), stop=(kt == 3))
        evict_kv(pkv)

    # attention (everything per-head at partition base 0)
    aoT = [apool.tile([128, N], BF16, tag="aoT", name="aoT", bufs=8) for _ in range(4)]
    for hp in range(4):
        for sub in range(2):
            if sub == 0:
                qh = qT[hp][0:64, :]
            else:
                qs = scrpool.tile([64, N], BF16, tag="qs", name="qs")
                nc.vector.tensor_copy(qs[:], qT[hp][64:128, :])
                qh = qs[:, :]
            pav = psA.tile([65, N], F32, tag="ps", name="ps")
            for jt, (joff, jsz) in enumerate(TOK_TILES):
                psim = psA.tile([128, N], F32, tag="ps", name="ps")
                nc.tensor.matmul(psim[:jsz, :], lhsT=kT[:, joff:joff + jsz],
                                 rhs=qh, start=True, stop=True)
                eT = apool.tile([128, N], BF16, tag="eT", name="eT", bufs=8)
                nc.scalar.activation(eT[:jsz, :], psim[:jsz, :], AF.Exp)
                if is_ptb:
                    nc.vector.tensor_tensor(eT[:jsz, :], eT[:jsz, :],
                                            p.c["maskm"][jt][:jsz, :],
                                            AluOpType.mult)
                nc.tensor.matmul(pav[:, :], lhsT=vaug[jt][:jsz, :],
                                 rhs=eT[:jsz, :], start=(jt == 0), stop=(jt == 1))
            rd = spool.tile([1, N], BF16, tag="rd", name="rd")
            with nc.allow_low_precision(reason="attn denom in bf16 for PE bcast"):
                nc.vector.reciprocal(rd[:], pav[64:65, :])
            pbc = psA.tile([64, N], F32, tag="ps", name="ps")
            nc.tensor.matmul(pbc[:, :], lhsT=p.c["ones1"][:1, :64], rhs=rd[:, :],
                             start=True, stop=True)
            araw = scrpool.tile([64, N], BF16, tag="araw", name="araw")
            nc.scalar.copy(araw[:], pav[0:64, :])
            if sub == 0:
                nc.vector.tensor_tensor(aoT[hp][0:64, :], araw[:], pbc[:],
                                        AluOpType.mult)
            else:
                aot2 = scrpool.tile([64, N], BF16, tag="aot2", name="aot2")
                nc.vector.tensor_tensor(aot2[:], araw[:], pbc[:], AluOpType.mult)
                nc.vector.tensor_copy(aoT[hp][64:128, :], aot2[:])

    # output projections + residual
    for tt, (toff, tsz) in enumerate(TOK_TILES):
        pd = psD.tile([128, D], F32, tag="psd", name="psd")
        for ht in range(4):
            nc.tensor.matmul(pd[:tsz, :], lhsT=aoT[ht][:, toff:toff + tsz],
                             rhs=wo[:, ht * D:(ht + 1) * D],
                             start=(ht == 0), stop=False)
        for ftile in range(13):
            nc.tensor.matmul(pd[:tsz, :], lhsT=swT[ftile][:, toff:toff + tsz],
                             rhs=wff[:, ftile * D:(ftile + 1) * D],
                             start=False, stop=(ftile == 12))
        xn = xpool.tile([128, D], F32, tag=f"x{tt}", name=f"x{tt}")
        nc.vector.tensor_tensor(xn[:tsz, :], pd[:tsz, :], p.x[tt][:tsz, :],
                                AluOpType.add)
        p.x[tt] = xn


def build_program(n_uni=4, n_mm=4, debug=False):
    nc = bass.Bass("TRN2", target_bir_lowering=False, debug=False,
                   num_devices=NC)
    dt_in = {}
    def din(name, shape, dt=BF16):
        dt_in[name] = nc.dram_tensor(name, shape, dt, kind="ExternalInput")
        return dt_in[name]

    t_im2 = din("im2col", [K27, S * NPOS])
    t_cw = din("convw", [K27, D])
    t_cb = din("convb", [128, 4], F32)
    t_cls = din("cls", [1, D], F32)
    t_id = din("ident", [128, 128])
    t_idf = din("identf", [128, 128], F32)
    t_rq = din("ropeq", [2, 128, N])
    t_rk = din("ropek", [2, 128, N])
    t_mm = din("maskm", [2, 128, N])
    t_lw = din("logitw", [1, 2 * D], F32)
    t_uwf = din("uni_wf", [4, 97, 4 * FUSED_PTB])
    t_uwo = din("uni_wo", [4, 128, 4 * D])
    t_uwff = din("uni_wff", [4, 128, 13 * D])
    t_mwf = din("mm_wf", [4, 97, 4 * FUSED_PTB])
    t_mwo = din("mm_wo", [4, 128, 4 * D])
    t_mwff = din("mm_wff", [4, 128, 13 * D])
    t_cwf = din("ca_wf", [4, 97, 4 * FUSED_CA])
    t_ckv = din("ca_wkv", [4, 97, 4 * 128])
    t_cwo = din("ca_wo", [4, 128, 4 * D])
    t_cwff = din("ca_wff2", [4, 128, 13 * D])

    t_out = nc.dram_tensor("logits", [1, 2], F32, kind="ExternalOutput")
    dbg = {}
    if debug:
        for nm in ("dbg_conv", "dbg_uni", "dbg_nrm", "dbg_fin"):
            dbg[nm] = nc.dram_tensor(nm, [N, D], F32, kind="ExternalOutput")
    xg_in = nc.dram_tensor("xg_in", [N, D], F32)
    xg_out = nc.dram_tensor("xg_out", [2, N, D], F32)

    with tile.TileContext(nc) as tc, ExitStack() as stk:
        cpool = stk.enter_context(tc.tile_pool(name="const", bufs=1))
        consts = {}
        for nm, t, shape, dt in (
            ("ident", t_id, [128, 128], BF16),
            ("identf", t_idf, [128, 128], F32),
            ("convw", t_cw, [K27, D], BF16),
            ("convb", t_cb, [128, 4], F32),
            ("logitw", t_lw, [1, 2 * D], F32),
        ):
            consts[nm] = cpool.tile(shape, dt, tag=nm, name=nm)
            nc.sync.dma_start(consts[nm][:], t[:, :])
        for nm, t in (("ropeq", t_rq), ("ropek", t_rk), ("maskm", t_mm)):
            pair = []
            for j in range(2):
                tt_ = cpool.tile([128, N], BF16, tag=f"{nm}{j}", name=f"{nm}{j}")
                nc.sync.dma_start(tt_[:], t[j, :, :])
                pair.append(tt_)
            consts[nm] = pair
        ones1 = cpool.tile([1, 64], BF16, tag="ones1", name="ones1")
        nc.vector.memset(ones1[:], 1.0)
        consts["ones1"] = ones1

        p = _Prog(nc, tc, consts)

        xpool = stk.enter_context(tc.tile_pool(name="x", bufs=3))
        p.x = [xpool.tile([128, D], F32, tag=f"x{tt}", name=f"x{tt}") for tt in range(2)]

        # ---------------- conv + pool phase
        with tc.tile_pool(name="conv", bufs=1) as convp, \
             tc.tile_pool(name="cscr", bufs=4) as cscr, \
             tc.tile_pool(name="cps", bufs=8, space="PSUM") as cps:
            im2 = convp.tile([K27, S * NPOS], BF16, tag="im2", name="im2")
            nc.sync.dma_start(im2[:], t_im2[:, :])
            zeros361 = convp.tile([128, NPOS], F32, tag="z361", name="z361")
            nc.vector.memset(zeros361[:], 0.0)
            pooledT = [convp.tile([97, S], F32, tag=f"pool{ft}", name=f"pool{ft}")
                       for ft in range(4)]
            for ft, (foff, fsz) in enumerate(D_TILES):
                for img in range(S):
                    ps = cps.tile([128, NPOS], F32, tag="cps", name="cps")
                    nc.tensor.matmul(
                        ps[:fsz, :], lhsT=consts["convw"][:, foff:foff + fsz],
                        rhs=im2[:, img * NPOS:(img + 1) * NPOS],
                        start=True, stop=True)
                    if ft < 2:
                        scr = cscr.tile([97, NPOS], BF16, tag="scr", name="scr")
                        nc.scalar.activation(
                            scr[:fsz, :], ps[:fsz, :], AF.Relu,
                            bias=consts["convb"][:fsz, ft:ft + 1],
                            accum_out=pooledT[ft][:fsz, img:img + 1])
                    else:
                        scr = cscr.tile([97, NPOS], BF16, tag="scr2", name="scr2")
                        nc.vector.scalar_tensor_tensor(
                            scr[:fsz, :], ps[:fsz, :],
                            consts["convb"][:fsz, ft:ft + 1],
                            zeros361[:fsz, :], AluOpType.add, AluOpType.max,
                            accum_out=pooledT[ft][:fsz, img:img + 1])
            # transpose pooled -> x tiles (fp32)
            for ft, (foff, fsz) in enumerate(D_TILES):
                for tt, (toff, tsz) in enumerate(IMG_TILES):
                    tp = cps.tile([128, NPOS], F32, tag="cps", name="cps")
                    nc.tensor.transpose(
                        tp[:tsz, :fsz], pooledT[ft][:fsz, toff:toff + tsz],
                        consts["identf"][:fsz, :fsz])
                    nc.any.tensor_copy(p.x[tt][:tsz, foff:foff + fsz],
                                       tp[:tsz, :fsz])
        nc.sync.dma_start(p.x[1][72:73, :], t_cls[:, :])

        if debug:
            for tt, (toff, tsz) in enumerate(TOK_TILES):
                nc.sync.dma_start(dbg["dbg_conv"][toff:toff + tsz, :],
                                  p.x[tt][:tsz, :])

        # ---------------- transformer pools
        wpool = stk.enter_context(tc.tile_pool(name="w", bufs=2))
        apool = stk.enter_context(tc.tile_pool(name="act", bufs=16))
        spool = stk.enter_context(tc.tile_pool(name="stats", bufs=8))
        scrpool = stk.enter_context(tc.tile_pool(name="scr", bufs=4))
        psA = stk.enter_context(tc.tile_pool(name="psA", bufs=4, space="PSUM"))
        psD = stk.enter_context(tc.tile_pool(name="psD", bufs=2, space="PSUM"))
        pools = (wpool, apool, spool, scrpool, psA, psD, xpool)

        for i in range(n_uni):
            _block(p, "ptb", t_uwf, t_uwo, t_uwff, i, pools)

        if debug:
            for tt, (toff, tsz) in enumerate(TOK_TILES):
                nc.sync.dma_start(dbg["dbg_uni"][toff:toff + tsz, :],
                                  p.x[tt][:tsz, :])

        # ---------------- pair exchange
        for tt, (toff, tsz) in enumerate(TOK_TILES):
            nc.sync.dma_start(xg_in[toff:toff + tsz, :], p.x[tt][:tsz, :])
        nc.gpsimd.collective_compute(
            "AllGather", AluOpType.bypass, replica_groups=REPLICA_PAIRS,
            ins=[xg_in.ap().opt()], outs=[xg_out.ap().opt()])
        nrmf = []
        for tt, (toff, tsz) in enumerate(TOK_TILES):
            xa = xpool.tile([128, D], F32, tag=f"x{tt}", name=f"x{tt}")
            nc.sync.dma_start(xa[:tsz, :], xg_out[0, toff:toff + tsz, :])
            p.x[tt] = xa
            nf = apool.tile([128, D], F32, tag="nrmf", name="nrmf", bufs=2)
            nc.sync.dma_start(nf[:tsz, :], xg_out[1, toff:toff + tsz, :])
            nrmf.append(nf)
        if debug:
            for tt, (toff, tsz) in enumerate(TOK_TILES):
                nc.sync.dma_start(dbg["dbg_nrm"][toff:toff + tsz, :],
                                  nrmf[tt][:tsz, :])
        nrmb = []
        for tt, (toff, tsz) in enumerate(TOK_TILES):
            nb = apool.tile([128, D], BF16, tag="nrmb", name="nrmb", bufs=2)
            nc.any.tensor_copy(nb[:tsz, :], nrmf[tt][:tsz, :])
            nrmb.append(nb)
        p.nrmT = _transpose_tiles(p, nrmb, apool, psA, "nrmT")

        # ---------------- mm + cross-attention phase
        for i in range(n_mm):
            _block(p, "ptb", t_mwf, t_mwo, t_mwff, i, pools)
            _block(p, "ca", t_cwf, t_cwo, t_cwff, i, pools, wkv_d=t_ckv)

        if debug:
            for tt, (toff, tsz) in enumerate(TOK_TILES):
                nc.sync.dma_start(dbg["dbg_fin"][toff:toff + tsz, :],
                                  p.x[tt][:tsz, :])

        # ---------------- logit head (fp32, on DVE)
        clsr = scrpool.tile([1, D], F32, tag="clsr", name="clsr")
        nc.sync.dma_start(clsr[:], p.x[1][72:73, :])
        cls_row = clsr
        m1 = spool.tile([1, 1], F32, tag="lg1", name="lg1")
        nc.vector.tensor_reduce(m1[:], cls_row[0:1, :], AX.X, AluOpType.add)
        nc.vector.tensor_scalar_mul(m1[:], m1[:], 1.0 / D)
        xm1 = scrpool.tile([1, D], F32, tag="lgxm", name="lgxm")
        nc.vector.tensor_scalar(xm1[:], cls_row[0:1, :], m1[:], None,
                                AluOpType.subtract)
        ssq1 = spool.tile([1, 1], F32, tag="lg2", name="lg2")
        scr1 = scrpool.tile([1, D], F32, tag="lgscr", name="lgscr")
        nc.scalar.activation(scr1[:], xm1[:], AF.Square, accum_out=ssq1[:])
        nc.vector.tensor_scalar(ssq1[:], ssq1[:], 1.0 / D, EPS,
                                AluOpType.mult, AluOpType.add)
        nc.scalar.activation(ssq1[:], ssq1[:], AF.Sqrt)
        rstd1 = spool.tile([1, 1], F32, tag="lg3", name="lg3")
        nc.vector.reciprocal(rstd1[:], ssq1[:])
        z1 = scrpool.tile([1, D], F32, tag="lgz", name="lgz")
        nc.vector.tensor_scalar(z1[:], xm1[:], rstd1[:], None, AluOpType.mult)
        lg = spool.tile([1, 2], F32, tag="lgout", name="lgout")
        for j in range(2):
            scrj = scrpool.tile([1, D], F32, tag="lgsc2", name="lgsc2")
            nc.vector.scalar_tensor_tensor(
                scrj[:], z1[:], 1.0, consts["logitw"][0:1, j * D:(j + 1) * D],
                AluOpType.mult, AluOpType.mult, accum_out=lg[:, j:j + 1])
        nc.sync.dma_start(t_out[:, :], lg[:])

    nc.finalize()
    _split_multi_waits(nc)
    return nc


# ---------------------------------------------------------------- entry
def kernel(**inputs) -> np.ndarray:
    key = "full"
    if key not in _COMPILED:
        _COMPILED[key] = build_program()
    nc = _COMPILED[key]
    in_maps = _host_prep(inputs)
    res = run_bass_kernel_spmd(nc, in_maps, core_ids=list(range(NC)))
    out = np.stack([res.results[2 * b]["logits"][0] for b in range(B)])
    return out.astype(np.float32)


if __name__ == "__main__":
    import sys
    sys.path.insert(0, "/root/problem")
    import reference
    inputs = {k: np.asarray(v) for k, v in reference.setup_inputs().items()}
    exp = np.asarray(reference.reference(**inputs))
    got = kernel(**inputs)
    err = np.abs(got - exp).max() / max(np.abs(exp).max(), 1e-9)
    print("expected:", exp)
    print("got:     ", got)
    print("Relative error:", err)



# revision 10
# speedup vs baseline: 1.3879x; 1.3879x over previous
"""CoCa image-tower kernel for 8 Trainium2 NeuronCores.

Strategy (SPMD, one program on all 8 cores):
  - core c = (batch b = c//2, chunk = c%2).  Each core conv-embeds its own
    200 images and runs the 4 "uni" transformer layers on its stream.
  - pair AllGather (cores 2b, 2b+1) exchanges the two streams, so both
    cores of a pair hold (a_b, nrm_b).
  - every core then runs the 4 (mm PTB + cross-attention) layers on a_b
    with nrm_b as context (pairs compute redundantly) and the logit head.
  - host: output row b is taken from core 2b.

Numerics: matmuls in bf16 (weights pre-cast/reordered on host, LN gains
absorbed into the following weight matrix), fp32 residual stream and
layer-norm statistics, fp32 PSUM accumulation everywhere.
"""

import numpy as np
import ml_dtypes

import concourse.bass as bass
import concourse.mybir as mybir
import concourse.tile as tile
from contextlib import ExitStack
from concourse.alu_op_type import AluOpType
from concourse.bass_utils import run_bass_kernel_spmd

AF = mybir.ActivationFunctionType
AX = mybir.AxisListType
BF16 = mybir.dt.bfloat16
F32 = mybir.dt.float32
NPBF = ml_dtypes.bfloat16

# ---------------------------------------------------------------- constants
B = 4
S = 200          # images per (batch, chunk)
D = 386
H = 8
DH = 64
FF = 1544
N = 201          # tokens (S + cls)
NC = 8
K27 = 27         # conv contraction (3 cin * 3x3)
K28 = 28         # + ones row carrying the conv bias
NPOS = 361       # 19*19 conv output positions
FFP = 1664       # ff width padded to 13*128
FUSED_PTB = 512 + 128 + FFP + FFP      # 3968 = 31*128
FUSED_CA = 512 + FFP + FFP             # 3840 = 30*128
D_TILES = [(0, 97), (97, 97), (194, 96), (290, 96)]
TOK_TILES = [(0, 128), (128, 73)]      # token tiling (incl cls)
IMG_TILES = [(0, 128), (128, 72)]      # image tiling (tokens w/o cls)
EPS = 1e-5
REPLICA_PAIRS = [[0, 1], [2, 3], [4, 5], [6, 7]]

_COMPILED = {}


# ---------------------------------------------------------------- host prep
def _im2col(imgs):
    """[n,3,40,40] f32 -> [28, n*361] bf16 rows (c, ky, kx) + ones row."""
    v = np.lib.stride_tricks.sliding_window_view(imgs, (3, 3), axis=(2, 3))
    v = v[:, :, ::2, ::2]                     # [n,3,19,19,3,3]
    v = v.transpose(1, 4, 5, 0, 2, 3).reshape(K27, -1)
    ones = np.ones((1, v.shape[1]), v.dtype)
    return np.ascontiguousarray(np.concatenate([v, ones], 0)).astype(NPBF)


def _ktile_cat(w):
    """[386, X] -> [97, 4*X]: D-tiles (97,97,96,96) side by side, zero pad."""
    X = w.shape[1]
    out = np.zeros((97, 4 * X), w.dtype)
    for i, (off, sz) in enumerate(D_TILES):
        out[:sz, i * X:(i + 1) * X] = w[off:off + sz]
    return out


def _rowtile_cat(w, p=128):
    """[R, X] -> [p, (R/p)*X]: row tiles side by side."""
    r, x = w.shape
    nt = r // p
    assert nt * p == r
    return np.concatenate([w[i * p:(i + 1) * p] for i in range(nt)], axis=1)


def _pad_ff_rows(w):
    """[1544, 386] -> [1664, 386] zero-padded rows."""
    return np.concatenate([w, np.zeros((FFP - FF, w.shape[1]), w.dtype)], 0)


def _prep_ptb(g, wf, wo, wff):
    wf = g[:, None] * wf
    q = wf[:, :512]
    kv = wf[:, 512:640]
    x1 = wf[:, 640:640 + FF]
    gate = wf[:, 640 + FF:]
    padc = np.zeros((D, FFP - FF), np.float32)
    wf2 = np.concatenate([q, kv, x1, padc, gate, padc], 1)
    assert wf2.shape[1] == FUSED_PTB
    return (
        _ktile_cat(wf2).astype(NPBF),
        _rowtile_cat(wo).astype(NPBF),
        _rowtile_cat(_pad_ff_rows(wff)).astype(NPBF),
    )


def _prep_ca(g, wq, wkv, wo, wf1, wf2_):
    wq = g[:, None] * wq * (DH ** -0.5)
    wf1 = g[:, None] * wf1
    h1 = wf1[:, :FF]
    gate = wf1[:, FF:]
    padc = np.zeros((D, FFP - FF), np.float32)
    fused = np.concatenate([wq, h1, padc, gate, padc], 1)
    assert fused.shape[1] == FUSED_CA
    return (
        _ktile_cat(fused).astype(NPBF),
        _ktile_cat(wkv).astype(NPBF),           # [97, 4*128]
        _rowtile_cat(wo).astype(NPBF),
        _rowtile_cat(_pad_ff_rows(wf2_)).astype(NPBF),
    )


def _rope_tables(scale):
    inv = 1.0 / (10000.0 ** (np.arange(0, DH, 2, dtype=np.float64) / DH))
    fr = np.arange(N, dtype=np.float64)[:, None] * inv[None, :]
    pos = np.concatenate([fr, fr], axis=-1)     # [N, 64]
    c = np.cos(pos).T * scale                   # [64, N]
    s = np.sin(pos).T * scale
    s_signed = np.concatenate([-s[:32], s[32:]], 0)
    ct = np.concatenate([c, c], 0)              # [128, N] (2 heads)
    st = np.concatenate([s_signed, s_signed], 0)
    return np.stack([ct, st]).astype(NPBF)      # [2, 128, N]


def _host_prep(inputs):
    """Build per-core in_maps from the raw reference inputs."""
    f32 = lambda a: np.asarray(a, np.float32)
    inp = {k: f32(v) for k, v in inputs.items()}

    # conv weights: W [28, 386] (rows c,ky,kx + bias row), pool scale folded in
    w27 = inp["conv_w"].transpose(1, 2, 3, 0).reshape(K27, D) / NPOS
    w28 = np.concatenate([w27, (inp["conv_b"] / NPOS)[None, :]], 0).astype(NPBF)
    shared = {
        "convw": w28,
        "cls": inp["cls_token"].reshape(1, D).astype(np.float32),
        "ident": np.eye(128, dtype=NPBF),
        "identf": np.eye(128, dtype=np.float32),
        "ropeq": _rope_tables(DH ** -0.5),
        "ropek": _rope_tables(1.0),
        "logitw": (inp["logit_ln"][:, None] * inp["logit_w"]).T.reshape(1, 2 * D).astype(np.float32),
    }
    # causal mask multiplier tiles [2, 128, N]
    mask = np.zeros((2, 128, N), np.float32)
    for jt, (joff, jsz) in enumerate(TOK_TILES):
        j = joff + np.arange(128)[:, None]
        mask[jt] = (j <= np.arange(N)[None, :]) & (j < N)
    shared["maskm"] = mask.astype(NPBF)

    for pre in ("uni", "mm"):
        wfs, wos, wffs = [], [], []
        for i in range(4):
            a, b, c = _prep_ptb(
                inp[f"{pre}_ln"][i], inp[f"{pre}_fused"][i],
                inp[f"{pre}_attn_out"][i], inp[f"{pre}_ff_out"][i])
            wfs.append(a); wos.append(b); wffs.append(c)
        shared[f"{pre}_wf"] = np.stack(wfs)
        shared[f"{pre}_wo"] = np.stack(wos)
        shared[f"{pre}_wff"] = np.stack(wffs)

    cfs, ckvs, cos_, cf2s = [], [], [], []
    for i in range(4):
        a, b, c, d = _prep_ca(
            inp["ca_ln"][i], inp["ca_q"][i], inp["ca_kv"][i],
            inp["ca_out"][i], inp["ca_ff1"][i], inp["ca_ff2"][i])
        cfs.append(a); ckvs.append(b); cos_.append(c); cf2s.append(d)
    shared["ca_wf"] = np.stack(cfs)
    shared["ca_wkv"] = np.stack(ckvs)
    shared["ca_wo"] = np.stack(cos_)
    shared["ca_wff2"] = np.stack(cf2s)

    in_maps = []
    x = inp["input"]  # [4, 400, 3, 40, 40]
    for c in range(NC):
        b, chunk = c // 2, c % 2
        imgs = x[b, chunk * S:(chunk + 1) * S]
        m = dict(shared)
        m["im2col"] = _im2col(imgs)
        in_maps.append(m)
    return in_maps


# ---------------------------------------------------------------- device IR
def _split_multi_waits(nc, max_waits=1):
    """walrus here rejects >1 semaphore wait per instruction; split extras
    onto single-wait NoOps preceding the instruction."""
    for f in nc.m.functions:
        for blk in f.blocks:
            new_instrs = []
            for ins in blk.instructions:
                si = getattr(ins, "sync_info", None)
                waits = list(si.on_wait) if si is not None and si.on_wait else []
                if len(waits) > max_waits:
                    keep = waits[-max_waits:]
                    for j, w in enumerate(waits[:-max_waits]):
                        new_instrs.append(mybir.InstNoOp(
                            name=f"{ins.name}_wsplit{j}", engine=ins.engine,
                            ins=[], outs=[],
                            sync_info=mybir.SyncInfo(on_wait=[w], on_update=[])))
                    si.on_wait = keep
                new_instrs.append(ins)
            blk.instructions = new_instrs


class _Prog:
    """Holds the Tile program state while building."""

    def __init__(self, nc, tc, consts):
        self.nc = nc
        self.tc = tc
        self.c = consts          # dict of const sbuf tiles
        self.x = [None, None]    # residual stream tiles [128,386] f32
        self.nrmT = None         # context, transposed bf16 tiles
        self.pools = {}


def _layernorm_z(p, apool, spool, scrpool, out_dtype=BF16):
    """LN (no gain) of p.x -> z tiles (bf16).  Returns z list."""
    nc = p.nc
    z = []
    for tt, (toff, tsz) in enumerate(TOK_TILES):
        xm = scrpool.tile([128, D], F32, tag="ln_xm", name="ln_xm")
        mean = spool.tile([128, 1], F32, tag="ln_st", name="ln_st")
        nc.vector.tensor_reduce(mean[:tsz], p.x[tt][:tsz, :], AX.X, AluOpType.add)
        nc.vector.tensor_scalar_mul(mean[:tsz], mean[:tsz], 1.0 / D)
        nc.vector.tensor_scalar(xm[:tsz, :], p.x[tt][:tsz, :], mean[:tsz], None,
                                AluOpType.subtract)
        ssq = spool.tile([128, 1], F32, tag="ln_st2", name="ln_st2")
        scr = scrpool.tile([128, D], BF16, tag="ln_scr", name="ln_scr")
        nc.scalar.activation(scr[:tsz, :], xm[:tsz, :], AF.Square,
                             accum_out=ssq[:tsz])
        sd = spool.tile([128, 1], F32, tag="ln_st3", name="ln_st3")
        nc.vector.tensor_scalar(sd[:tsz], ssq[:tsz], 1.0 / D, EPS,
                                AluOpType.mult, AluOpType.add)
        nc.scalar.activation(sd[:tsz], sd[:tsz], AF.Sqrt)
        rstd = spool.tile([128, 1], F32, tag="ln_st4", name="ln_st4")
        nc.vector.reciprocal(rstd[:tsz], sd[:tsz])
        zt = apool.tile([128, D], out_dtype, tag="z", name="z", bufs=4)
        nc.vector.tensor_scalar(zt[:tsz, :], xm[:tsz, :], rstd[:tsz], None,
                                AluOpType.mult)
        z.append(zt)
    return z


def _transpose_tiles(p, src_tiles, apool, psA, tag, src_rows=TOK_TILES):
    """src [tok,386] bf16 tiles -> 4 x [97, N] bf16 transposed tiles."""
    nc = p.nc
    out = [apool.tile([97, N], BF16, tag=tag, name=tag, bufs=8 if tag == "zT" else 4) for _ in range(4)]
    for kt, (koff, ksz) in enumerate(D_TILES):
        for tt, (toff, tsz) in enumerate(src_rows):
            tp = psA.tile([128, 128], BF16, tag="psb", name="psb", bufs=2)
            nc.tensor.transpose(tp[:ksz, :tsz], src_tiles[tt][:tsz, koff:koff + ksz],
                                p.c["ident"][:tsz, :tsz])
            nc.any.tensor_copy(out[kt][:ksz, toff:toff + tsz], tp[:ksz, :tsz])
    return out


def _rope(p, raw, table, nrows, scrpool, out_tile):
    """Apply rotary embedding in [dh, tok] layout (all operands base 0).

    raw: sbuf bf16 [128, N] tile (rows = head dims, 64 per head);
    table: const [2,128,N] (cos, sin_signed); nrows: 64 or 128.
    Engines only allow a partition shift on single-input copies, so build
    the rotate-half shuffle with copies, then same-partition muls/adds."""
    nc = p.nc
    ct, st = table
    sh = scrpool.tile([128, N], BF16, tag="ropesh", name="ropesh")
    for h0 in range(0, nrows, 64):
        nc.vector.tensor_copy(sh[h0:h0 + 32, :], raw[h0 + 32:h0 + 64, :])
        nc.vector.tensor_copy(sh[h0 + 32:h0 + 64, :], raw[h0:h0 + 32, :])
    t1 = scrpool.tile([128, N], BF16, tag="rope1", name="rope1")
    nc.vector.tensor_tensor(t1[:nrows, :], raw[:nrows, :], ct[:nrows, :],
                            AluOpType.mult)
    nc.vector.tensor_tensor(sh[:nrows, :], sh[:nrows, :], st[:nrows, :],
                            AluOpType.mult)
    nc.vector.tensor_tensor(out_tile[:nrows, :], t1[:nrows, :],
                            sh[:nrows, :], AluOpType.add)


def _block(p, kind, wf_d, wo_d, wff_d, layer, pools, wkv_d=None):
    """One transformer block (PTB or CA), updates p.x in place."""
    nc = p.nc
    (wpool, apool, spool, scrpool, psA, psD, xpool) = pools
    is_ptb = kind == "ptb"
    fused_w = FUSED_PTB if is_ptb else FUSED_CA
    n_m = fused_w // 128

    wf = wpool.tile([97, 4 * FUSED_PTB], BF16, tag="wf", name="wf")
    nc.sync.dma_start(wf[:, :4 * fused_w], wf_d[layer, :, :])
    wo = wpool.tile([128, 4 * D], BF16, tag="wo", name="wo")
    nc.sync.dma_start(wo[:], wo_d[layer, :, :])
    wff = wpool.tile([128, 13 * D], BF16, tag="wff", name="wff")
    nc.sync.dma_start(wff[:], wff_d[layer, :, :])
    if not is_ptb:
        wkv = wpool.tile([97, 4 * 128], BF16, tag="wkv", name="wkv")
        nc.sync.dma_start(wkv[:], wkv_d[layer, :, :])

    z = _layernorm_z(p, apool, spool, scrpool)
    zT = _transpose_tiles(p, z, apool, psA, "zT")

    qT = [apool.tile([128, N], BF16, tag="qT", name="qT", bufs=8) for _ in range(4)]
    kT = apool.tile([64, N], BF16, tag="kT", name="kT", bufs=2)
    vaug = [apool.tile([128, 65], BF16, tag="vaug", name="vaug", bufs=4) for _ in range(2)]
    x1T = [apool.tile([128, N], BF16, tag="x1T", name="x1T", bufs=14) for _ in range(13)]
    swT = [apool.tile([128, N], BF16, tag="swT", name="swT", bufs=14) for _ in range(13)]

    def evict_kv(pf):
        kvraw = scrpool.tile([128, N], BF16, tag="kvraw", name="kvraw")
        nc.scalar.copy(kvraw[:], pf[:])
        if is_ptb:
            _rope(p, kvraw, (p.c["ropek"][0], p.c["ropek"][1]), 64, scrpool, kT)
        else:
            nc.any.tensor_copy(kT[:], kvraw[:64, :])
        vsb = scrpool.tile([64, N], BF16, tag="vsb", name="vsb")
        nc.vector.tensor_copy(vsb[:], kvraw[64:128, :])
        for jt, (joff, jsz) in enumerate(TOK_TILES):
            tv = psA.tile([128, 128], BF16, tag="psb", name="psb", bufs=2)
            nc.tensor.transpose(tv[:jsz, :64], vsb[:, joff:joff + jsz],
                                p.c["ident"][:64, :64])
            nc.any.tensor_copy(vaug[jt][:jsz, :64], tv[:jsz, :64])
            nc.vector.memset(vaug[jt][:jsz, 64:65], 1.0)

    # fused projection (stationary = weight tile, moving = zT)
    x1_base = 5 if is_ptb else 4
    gate_base = 18 if is_ptb else 17
    for m in range(n_m):
        pf = psA.tile([128, N], F32, tag="ps", name="ps")
        for kt, (koff, ksz) in enumerate(D_TILES):
            nc.tensor.matmul(pf[:, :],
                             lhsT=wf[:ksz, kt * fused_w + m * 128:
                                     kt * fused_w + (m + 1) * 128],
                             rhs=zT[kt][:ksz, :],
                             start=(kt == 0), stop=(kt == 3))
        if m < 4:  # q heads
            if is_ptb:
                qraw = scrpool.tile([128, N], BF16, tag="qraw", name="qraw")
                nc.scalar.copy(qraw[:], pf[:])
                _rope(p, qraw, (p.c["ropeq"][0], p.c["ropeq"][1]), 128, scrpool,
                      qT[m])
            else:
                nc.scalar.copy(qT[m][:], pf[:])
        elif is_ptb and m == 4:
            evict_kv(pf)
        elif x1_base <= m < gate_base:
            nc.scalar.copy(x1T[m - x1_base][:], pf[:])
        else:
            j = m - gate_base
            gs = scrpool.tile([128, N], BF16, tag="gsil", name="gsil")
            nc.scalar.activation(gs[:], pf[:], AF.Silu)
            nc.vector.tensor_tensor(swT[j][:], gs[:], x1T[j][:], AluOpType.mult)

    if not is_ptb:  # kv from context
        pkv = psA.tile([128, N], F32, tag="ps", name="ps")
        for kt, (koff, ksz) in enumerate(D_TILES):
            nc.tensor.matmul(pkv[:, :],
                             lhsT=wkv[:ksz, kt * 128:(kt + 1) * 128],
                             rhs=p.nrmT[kt][:ksz, :],
                             start=(kt == 0), stop=(kt == 3))
        evict_kv(pkv)

    # attention (everything per-head at partition base 0)
    aoT = [apool.tile([128, N], BF16, tag="aoT", name="aoT", bufs=8) for _ in range(4)]
    for hp in range(4):
        for sub in range(2):
            if sub == 0:
                qh = qT[hp][0:64, :]
            else:
                qs = scrpool.tile([64, N], BF16, tag="qs", name="qs")
                nc.vector.tensor_copy(qs[:], qT[hp][64:128, :])
                qh = qs[:, :]
            pav = psA.tile([65, N], F32, tag="ps", name="ps")
            for jt, (joff, jsz) in enumerate(TOK_TILES):
                psim = psA.tile([128, N], F32, tag="ps", name="ps")
                nc.tensor.matmul(psim[:jsz, :], lhsT=kT[:, joff:joff + jsz],
                                 rhs=qh, start=True, stop=True)
                eT = apool.tile([128, N], BF16, tag="eT", name="eT", bufs=8)
                nc.scalar.activation(eT[:jsz, :], psim[:jsz, :], AF.Exp)
                if is_ptb:
                    nc.vector.tensor_tensor(eT[:jsz, :], eT[:jsz, :],
                                            p.c["maskm"][jt][:jsz, :],
                                            AluOpType.mult)
                nc.tensor.matmul(pav[:, :], lhsT=vaug[jt][:jsz, :],
                                 rhs=eT[:jsz, :], start=(jt == 0), stop=(jt == 1))
            rd = spool.tile([1, N], BF16, tag="rd", name="rd")
            with nc.allow_low_precision(reason="attn denom in bf16 for PE bcast"):
                nc.vector.reciprocal(rd[:], pav[64:65, :])
            pbc = psA.tile([64, N], F32, tag="ps", name="ps")
            nc.tensor.matmul(pbc[:, :], lhsT=p.c["ones1"][:1, :64], rhs=rd[:, :],
                             start=True, stop=True)
            araw = scrpool.tile([64, N], BF16, tag="araw", name="araw")
            nc.scalar.copy(araw[:], pav[0:64, :])
            if sub == 0:
                nc.vector.tensor_tensor(aoT[hp][0:64, :], araw[:], pbc[:],
                                        AluOpType.mult)
            else:
                aot2 = scrpool.tile([64, N], BF16, tag="aot2", name="aot2")
                nc.vector.tensor_tensor(aot2[:], araw[:], pbc[:], AluOpType.mult)
                nc.vector.tensor_copy(aoT[hp][64:128, :], aot2[:])

    # output projections + residual
    for tt, (toff, tsz) in enumerate(TOK_TILES):
        pd = psD.tile([128, D], F32, tag="psd", name="psd")
        for ht in range(4):
            nc.tensor.matmul(pd[:tsz, :], lhsT=aoT[ht][:, toff:toff + tsz],
                             rhs=wo[:, ht * D:(ht + 1) * D],
                             start=(ht == 0), stop=False)
        for ftile in range(13):
            nc.tensor.matmul(pd[:tsz, :], lhsT=swT[ftile][:, toff:toff + tsz],
                             rhs=wff[:, ftile * D:(ftile + 1) * D],
                             start=False, stop=(ftile == 12))
        xn = xpool.tile([128, D], F32, tag=f"x{tt}", name=f"x{tt}")
        nc.vector.tensor_tensor(xn[:tsz, :], pd[:tsz, :], p.x[tt][:tsz, :],
                                AluOpType.add)
        p.x[tt] = xn


def build_program(n_uni=4, n_mm=4, debug=False):
    nc = bass.Bass("TRN2", target_bir_lowering=False, debug=False,
                   num_devices=NC)
    dt_in = {}
    def din(name, shape, dt=BF16):
        dt_in[name] = nc.dram_tensor(name, shape, dt, kind="ExternalInput")
        return dt_in[name]

    t_im2 = din("im2col", [K28, S * NPOS])
    t_cw = din("convw", [K28, D])
    t_cls = din("cls", [1, D], F32)
    t_id = din("ident", [128, 128])
    t_idf = din("identf", [128, 128], F32)
    t_rq = din("ropeq", [2, 128, N])
    t_rk = din("ropek", [2, 128, N])
    t_mm = din("maskm", [2, 128, N])
    t_lw = din("logitw", [1, 2 * D], F32)
    t_uwf = din("uni_wf", [4, 97, 4 * FUSED_PTB])
    t_uwo = din("uni_wo", [4, 128, 4 * D])
    t_uwff = din("uni_wff", [4, 128, 13 * D])
    t_mwf = din("mm_wf", [4, 97, 4 * FUSED_PTB])
    t_mwo = din("mm_wo", [4, 128, 4 * D])
    t_mwff = din("mm_wff", [4, 128, 13 * D])
    t_cwf = din("ca_wf", [4, 97, 4 * FUSED_CA])
    t_ckv = din("ca_wkv", [4, 97, 4 * 128])
    t_cwo = din("ca_wo", [4, 128, 4 * D])
    t_cwff = din("ca_wff2", [4, 128, 13 * D])

    t_out = nc.dram_tensor("logits", [1, 2], F32, kind="ExternalOutput")
    dbg = {}
    if debug:
        for nm in ("dbg_conv", "dbg_uni", "dbg_nrm", "dbg_fin"):
            dbg[nm] = nc.dram_tensor(nm, [N, D], F32, kind="ExternalOutput")
    xg_in = nc.dram_tensor("xg_in", [N, D], F32)
    xg_out = nc.dram_tensor("xg_out", [2, N, D], F32)

    with tile.TileContext(nc) as tc, ExitStack() as stk:
        cpool = stk.enter_context(tc.tile_pool(name="const", bufs=1))
        consts = {}
        for nm, t, shape, dt in (
            ("ident", t_id, [128, 128], BF16),
            ("identf", t_idf, [128, 128], F32),
            ("convw", t_cw, [K28, D], BF16),
            ("logitw", t_lw, [1, 2 * D], F32),
        ):
            consts[nm] = cpool.tile(shape, dt, tag=nm, name=nm)
            nc.sync.dma_start(consts[nm][:], t[:, :])
        for nm, t in (("ropeq", t_rq), ("ropek", t_rk), ("maskm", t_mm)):
            pair = []
            for j in range(2):
                tt_ = cpool.tile([128, N], BF16, tag=f"{nm}{j}", name=f"{nm}{j}")
                nc.sync.dma_start(tt_[:], t[j, :, :])
                pair.append(tt_)
            consts[nm] = pair
        ones1 = cpool.tile([1, 64], BF16, tag="ones1", name="ones1")
        nc.vector.memset(ones1[:], 1.0)
        consts["ones1"] = ones1

        p = _Prog(nc, tc, consts)

        xpool = stk.enter_context(tc.tile_pool(name="x", bufs=3))
        p.x = [xpool.tile([128, D], F32, tag=f"x{tt}", name=f"x{tt}") for tt in range(2)]

        # ---------------- conv + pool phase
        # relu+pool split across Act / DVE / Pool, weighted by per-op cost
        # (Act 631ns: psum access + accum read; DVE/Pool ~501ns).
        sched = []
        credit = {"A": 0.0, "D": 0.0, "P": 0.0}
        rate = {"A": 1 / 631.0, "D": 1 / 501.0, "P": 1 / 501.0}
        for _ in range(4 * S):
            for k in credit:
                credit[k] += rate[k]
            pick = max(credit, key=lambda k: credit[k])
            credit[pick] -= rate["A"] + rate["D"] + rate["P"]
            sched.append(pick)
        with tc.tile_pool(name="conv", bufs=1) as convp, \
             tc.tile_pool(name="cscr", bufs=4) as cscr, \
             tc.tile_pool(name="cps", bufs=8, space="PSUM") as cps:
            im2 = convp.tile([K28, S * NPOS], BF16, tag="im2", name="im2")
            nc.sync.dma_start(im2[:], t_im2[:, :])
            pooledT = [convp.tile([97, S], F32, tag=f"pool{ft}", name=f"pool{ft}")
                       for ft in range(4)]
            si = 0
            for ft, (foff, fsz) in enumerate(D_TILES):
                for img in range(S):
                    ps = cps.tile([128, NPOS], F32, tag="cps", name="cps")
                    nc.tensor.matmul(
                        ps[:fsz, :], lhsT=consts["convw"][:, foff:foff + fsz],
                        rhs=im2[:, img * NPOS:(img + 1) * NPOS],
                        start=True, stop=True)
                    eng = sched[si]; si += 1
                    scr = cscr.tile([97, NPOS], BF16, tag=f"scr{eng}",
                                    name=f"scr{eng}", bufs=3)
                    acc = pooledT[ft][:fsz, img:img + 1]
                    if eng == "A":
                        nc.scalar.activation(scr[:fsz, :], ps[:fsz, :], AF.Relu,
                                             accum_out=acc)
                    elif eng == "D":
                        nc.vector.tensor_scalar(scr[:fsz, :], ps[:fsz, :], 0.0,
                                                None, AluOpType.max,
                                                accum_out=acc)
                    else:
                        nc.gpsimd.tensor_scalar(scr[:fsz, :], ps[:fsz, :], 0.0,
                                                None, AluOpType.max,
                                                accum_out=acc)
            # transpose pooled -> x tiles (fp32)
            for ft, (foff, fsz) in enumerate(D_TILES):
                for tt, (toff, tsz) in enumerate(IMG_TILES):
                    tp = cps.tile([128, NPOS], F32, tag="cps", name="cps")
                    nc.tensor.transpose(
                        tp[:tsz, :fsz], pooledT[ft][:fsz, toff:toff + tsz],
                        consts["identf"][:fsz, :fsz])
                    nc.any.tensor_copy(p.x[tt][:tsz, foff:foff + fsz],
                                       tp[:tsz, :fsz])
        nc.sync.dma_start(p.x[1][72:73, :], t_cls[:, :])

        if debug:
            for tt, (toff, tsz) in enumerate(TOK_TILES):
                nc.sync.dma_start(dbg["dbg_conv"][toff:toff + tsz, :],
                                  p.x[tt][:tsz, :])

        # ---------------- transformer pools
        wpool = stk.enter_context(tc.tile_pool(name="w", bufs=2))
        apool = stk.enter_context(tc.tile_pool(name="act", bufs=16))
        spool = stk.enter_context(tc.tile_pool(name="stats", bufs=8))
        scrpool = stk.enter_context(tc.tile_pool(name="scr", bufs=4))
        psA = stk.enter_context(tc.tile_pool(name="psA", bufs=4, space="PSUM"))
        psD = stk.enter_context(tc.tile_pool(name="psD", bufs=2, space="PSUM"))
        pools = (wpool, apool, spool, scrpool, psA, psD, xpool)

        for i in range(n_uni):
            _block(p, "ptb", t_uwf, t_uwo, t_uwff, i, pools)

        if debug:
            for tt, (toff, tsz) in enumerate(TOK_TILES):
                nc.sync.dma_start(dbg["dbg_uni"][toff:toff + tsz, :],
                                  p.x[tt][:tsz, :])

        # ---------------- pair exchange (overlapped with first mm block)
        # p.x stays the local stream: on even cores that's a_b (the one whose
        # logits are collected); odd cores compute redundantly on nrm_b and
        # their output is ignored.  Only the partner context xg_out[1] is read
        # back, and only right before the first cross-attention block.
        for tt, (toff, tsz) in enumerate(TOK_TILES):
            nc.sync.dma_start(xg_in[toff:toff + tsz, :], p.x[tt][:tsz, :])
        nc.gpsimd.collective_compute(
            "AllGather", AluOpType.bypass, replica_groups=REPLICA_PAIRS,
            ins=[xg_in.ap().opt()], outs=[xg_out.ap().opt()])

        # ---------------- mm + cross-attention phase
        for i in range(n_mm):
            _block(p, "ptb", t_mwf, t_mwo, t_mwff, i, pools)
            if i == 0:
                nrmf = []
                for tt, (toff, tsz) in enumerate(TOK_TILES):
                    nf = apool.tile([128, D], F32, tag="nrmf", name="nrmf", bufs=2)
                    nc.sync.dma_start(nf[:tsz, :], xg_out[1, toff:toff + tsz, :])
                    nrmf.append(nf)
                if debug:
                    for tt, (toff, tsz) in enumerate(TOK_TILES):
                        nc.sync.dma_start(dbg["dbg_nrm"][toff:toff + tsz, :],
                                          nrmf[tt][:tsz, :])
                nrmb = []
                for tt, (toff, tsz) in enumerate(TOK_TILES):
                    nb = apool.tile([128, D], BF16, tag="nrmb", name="nrmb", bufs=2)
                    nc.any.tensor_copy(nb[:tsz, :], nrmf[tt][:tsz, :])
                    nrmb.append(nb)
                p.nrmT = _transpose_tiles(p, nrmb, apool, psA, "nrmT")
            _block(p, "ca", t_cwf, t_cwo, t_cwff, i, pools, wkv_d=t_ckv)

        if debug:
            for tt, (toff, tsz) in enumerate(TOK_TILES):
                nc.sync.dma_start(dbg["dbg_fin"][toff:toff + tsz, :],
                                  p.x[tt][:tsz, :])

        # ---------------- logit head (fp32, on DVE)
        clsr = scrpool.tile([1, D], F32, tag="clsr", name="clsr")
        nc.sync.dma_start(clsr[:], p.x[1][72:73, :])
        cls_row = clsr
        m1 = spool.tile([1, 1], F32, tag="lg1", name="lg1")
        nc.vector.tensor_reduce(m1[:], cls_row[0:1, :], AX.X, AluOpType.add)
        nc.vector.tensor_scalar_mul(m1[:], m1[:], 1.0 / D)
        xm1 = scrpool.tile([1, D], F32, tag="lgxm", name="lgxm")
        nc.vector.tensor_scalar(xm1[:], cls_row[0:1, :], m1[:], None,
                                AluOpType.subtract)
        ssq1 = spool.tile([1, 1], F32, tag="lg2", name="lg2")
        scr1 = scrpool.tile([1, D], F32, tag="lgscr", name="lgscr")
        nc.scalar.activation(scr1[:], xm1[:], AF.Square, accum_out=ssq1[:])
        nc.vector.tensor_scalar(ssq1[:], ssq1[:], 1.0 / D, EPS,
                                AluOpType.mult, AluOpType.add)
        nc.scalar.activation(ssq1[:], ssq1[:], AF.Sqrt)
        rstd1 = spool.tile([1, 1], F32, tag="lg3", name="lg3")
        nc.vector.reciprocal(rstd1[:], ssq1[:])
        z1 = scrpool.tile([1, D], F32, tag="lgz", name="lgz")
        nc.vector.tensor_scalar(z1[:], xm1[:], rstd1[:], None, AluOpType.mult)
        lg = spool.tile([1, 2], F32, tag="lgout", name="lgout")
        for j in range(2):
            scrj = scrpool.tile([1, D], F32, tag="lgsc2", name="lgsc2")
            nc.vector.scalar_tensor_tensor(
                scrj[:], z1[:], 1.0, consts["logitw"][0:1, j * D:(j + 1) * D],
                AluOpType.mult, AluOpType.mult, accum_out=lg[:, j:j + 1])
        nc.sync.dma_start(t_out[:, :], lg[:])

    nc.finalize()
    _split_multi_waits(nc)
    return nc


# ---------------------------------------------------------------- entry
def kernel(**inputs) -> np.ndarray:
    key = "full"
    if key not in _COMPILED:
        _COMPILED[key] = build_program()
    nc = _COMPILED[key]
    in_maps = _host_prep(inputs)
    res = run_bass_kernel_spmd(nc, in_maps, core_ids=list(range(NC)))
    out = np.stack([res.results[2 * b]["logits"][0] for b in range(B)])
    return out.astype(np.float32)


if __name__ == "__main__":
    import sys
    sys.path.insert(0, "/root/problem")
    import reference
    inputs = {k: np.asarray(v) for k, v in reference.setup_inputs().items()}
    exp = np.asarray(reference.reference(**inputs))
    got = kernel(**inputs)
    err = np.abs(got - exp).max() / max(np.abs(exp).max(), 1e-9)
    print("expected:", exp)
    print("got:     ", got)
    print("Relative error:", err)

